# revision 16
# baseline (speedup 1.0000x reference)
"""Multi-head causal attention (B=4, S=2048, D=1024, H=16) on 8 NeuronCores.

Sharding: batch x head-group. Core c handles batch c//2 and heads
[8*(c%2), 8*(c%2)+8). Wq/Wk/Wv are split column-wise by head (512 output
features per core), Wo row-wise; the 2-way partial-sum reduction over
head-groups (+ bias) is done on the host after gathering per-core outputs.

Per-core dataflow (all matmul operands bf16, fp32 PSUM accumulation):
  QT[e,s], KT[e,s] = Wslice^T-stationary x^T-moving matmuls (transposed layouts)
  V[s,e] natural layout, augmented with a ones column per head (row sums)
  scoresT[k,q] = KT-slice stationary, QT moving  (K=64 contraction per head)
  pT = exp(scoresT/8)  (no max subtraction: logits are bounded ~|2.5|)
  causal masking via gpsimd affine_select on diagonal tiles (fill 0 post-exp)
  ctx^T[hd,q] (+ row-sum l in row 64) = Vaug stationary, pT moving
  normalize by 1/l (DMA partition-broadcast of reciprocal), out-proj with Wo^T.
"""

import contextlib

import numpy as np
import ml_dtypes

S = 2048
D = 1024
E = 512       # per-core output features of Q/K/V projections (8 heads x 64)
HL = 8        # local heads
HD = 64
DC = D // 128   # 8 contraction chunks for projections
ET = E // 128   # 4 chunks for QT/KT/ctxn
NSB = S // 512  # 4 s-blocks
NST = S // 128  # 16 s-tiles
NKT = S // 128  # 16 k-tiles

_NC_CACHE = {}


def _build_nc():
    import concourse.tile as tile
    from concourse import bacc, mybir

    f32 = mybir.dt.float32
    bf16 = mybir.dt.bfloat16
    Exp = mybir.ActivationFunctionType.Exp

    nc = bacc.Bacc("TRN2", target_bir_lowering=False, debug=False, enable_asserts=False)
    xT = nc.dram_tensor("xT", [D, S], bf16, kind="ExternalInput").ap()
    wqT = nc.dram_tensor("wqT", [D, E], bf16, kind="ExternalInput").ap()
    wkT = nc.dram_tensor("wkT", [D, E], bf16, kind="ExternalInput").ap()
    wvT = nc.dram_tensor("wvT", [D, E], bf16, kind="ExternalInput").ap()
    woT = nc.dram_tensor("woT", [E, D], bf16, kind="ExternalInput").ap()
    out = nc.dram_tensor("out", [S, D], f32, kind="ExternalOutput").ap()

    with tile.TileContext(nc) as tc, contextlib.ExitStack() as ctx:
        const = ctx.enter_context(tc.tile_pool(name="const", bufs=1))

        xT_sb = [const.tile([128, S], bf16, tag=f"xT{c}", name=f"xT{c}") for c in range(DC)]
        wq_sb = [const.tile([128, E], bf16, tag=f"wq{c}", name=f"wq{c}") for c in range(DC)]
        wk_sb = [const.tile([128, E], bf16, tag=f"wk{c}", name=f"wk{c}") for c in range(DC)]
        wv_sb = [const.tile([128, E], bf16, tag=f"wv{c}", name=f"wv{c}") for c in range(DC)]
        wo_sb = [const.tile([128, D], bf16, tag=f"wo{c}", name=f"wo{c}") for c in range(ET)]
        qT_sb = [const.tile([128, S], bf16, tag=f"qT{c}", name=f"qT{c}") for c in range(ET)]
        # K^T in two zero-padded parity variants: kTe has the odd head's rows
        # zeroed (and vice versa) so scores matmuls contract over K=128
        # partitions (K=64 matmuls stream at half rate on TRN2).
        kTe_sb = [const.tile([128, S], bf16, tag=f"kTe{c}", name=f"kTe{c}") for c in range(ET)]
        kTo_sb = [const.tile([128, S], bf16, tag=f"kTo{c}", name=f"kTo{c}") for c in range(ET)]
        # V augmented: per s-tile, 8 heads x (64 V cols + 1 ones col)
        va_sb = [const.tile([128, HL * (HD + 1)], bf16, tag=f"va{t}", name=f"va{t}")
                 for t in range(NST)]
        ctxn_sb = [const.tile([128, S], bf16, tag=f"cn{c}", name=f"cn{c}") for c in range(ET)]

        # load order tracks first use: xT s-block 0 and wq feed the very first
        # projection group; wo is needed last
        for c in range(DC):
            nc.sync.dma_start(xT_sb[c][:, 0:512], xT[128 * c:128 * (c + 1), 0:512])
        for c in range(DC):
            nc.sync.dma_start(wq_sb[c][:], wqT[128 * c:128 * (c + 1), :])
        for c in range(DC):
            nc.sync.dma_start(wk_sb[c][:], wkT[128 * c:128 * (c + 1), :])
        for sb in range(1, NSB):
            for c in range(DC):
                nc.sync.dma_start(xT_sb[c][:, 512 * sb:512 * (sb + 1)],
                                  xT[128 * c:128 * (c + 1), 512 * sb:512 * (sb + 1)])
        for c in range(DC):
            nc.sync.dma_start(wv_sb[c][:], wvT[128 * c:128 * (c + 1), :])
        for c in range(ET):
            nc.sync.dma_start(wo_sb[c][:], woT[128 * c:128 * (c + 1), :])
        for t in range(NST):
            # ones columns at 64 + 65*k within each 130-wide head pair group
            va4 = va_sb[t].rearrange("p (g k c) -> p g k c", k=2, c=HD + 1)
            nc.vector.memset(va4[:, :, :, HD:HD + 1], 1.0)
        for c in range(ET):
            # zero the dead halves once; phase 1 fills only the live halves
            nc.vector.memset(kTe_sb[c][64:128, :], 0.0)
            nc.vector.memset(kTo_sb[c][0:64, :], 0.0)

        # One PSUM pool for all phases: tag "sc" slots ([128,1024] = 2 banks,
        # bufs=2) are shared by projection, scores, and out-projection tiles;
        # tags ctx0..3 hold the per-q-block PV accumulators (1 bank each).
        # 2*2 + 4 = 8 banks, no pool-boundary serialization between phases.
        psum = ctx.enter_context(tc.tile_pool(name="psum", bufs=1, space="PSUM"))
        ptp = ctx.enter_context(tc.tile_pool(name="ptp", bufs=8))
        csp = ctx.enter_context(tc.tile_pool(name="csp", bufs=8))
        epp = ctx.enter_context(tc.tile_pool(name="epp", bufs=2))
        bcp = ctx.enter_context(tc.tile_pool(name="bcp", bufs=3))
        ldp = ctx.enter_context(tc.tile_pool(name="ldp", bufs=2, space="DRAM"))
        stp = ctx.enter_context(tc.tile_pool(name="stp", bufs=3))
        otp = ctx.enter_context(tc.tile_pool(name="otp", bufs=4))

        # ---- Phase 1: Q^T, K^T, V projections ----
        for sb in range(NSB):
            for w_sb, is_k in ((wq_sb, False), (wk_sb, True)):
                for et in range(ET):
                    ps = psum.tile([128, 512], f32, tag="sc", bufs=2,
                                   name=f"ps{sb}_{et}_{int(is_k)}")
                    for dc in range(DC):
                        nc.tensor.matmul(
                            ps[:],
                            w_sb[dc][:, 128 * et:128 * (et + 1)],
                            xT_sb[dc][:, 512 * sb:512 * (sb + 1)],
                            start=(dc == 0), stop=(dc == DC - 1))
                    ssl = slice(512 * sb, 512 * (sb + 1))
                    if is_k:
                        nc.vector.tensor_copy(kTe_sb[et][0:64, ssl], ps[0:64, :])
                        nc.vector.tensor_copy(kTo_sb[et][64:128, ssl], ps[64:128, :])
                    else:
                        nc.vector.tensor_copy(qT_sb[et][:, ssl], ps[:])
            for st4 in range(4):
                st = 4 * sb + st4
                ps = psum.tile([128, 512], f32, tag="sc", bufs=2, name=f"pv{st}")
                for dc in range(DC):
                    nc.tensor.matmul(
                        ps[:],
                        xT_sb[dc][:, 128 * st:128 * (st + 1)],
                        wv_sb[dc][:],
                        start=(dc == 0), stop=(dc == DC - 1))
                va = va_sb[st].rearrange("p (g pair) -> p g pair", pair=2 * (HD + 1))
                psg = ps.rearrange("p (g c) -> p g c", c=2 * HD)
                nc.vector.tensor_copy(va[:, :, 0:HD], psg[:, :, 0:HD])
                nc.vector.tensor_copy(va[:, :, HD + 1:2 * HD + 1], psg[:, :, HD:2 * HD])

        # ---- Phase 2: attention per head ----
        for h in range(HL):
            chk = h // 2
            kc = (kTo_sb if h % 2 else kTe_sb)[chk]
            qc = qT_sb[chk]
            ctx_t = [psum.tile([65, 512], f32, tag=f"ctx{j}", name=f"ctx_h{h}_{j}")
                     for j in range(4)]
            cs = [csp.tile([65, 512], f32, tag="cs", name=f"cs_h{h}_{j}")
                  for j in range(4)]
            lr = epp.tile([128, 16], f32, tag="lr", name=f"lr{h}")
            # software pipeline: PV matmuls for a chunk are emitted after the
            # NEXT chunk's scores, so the in-order PE queue has score work to
            # run while ACT computes the exp the PV depends on
            pending_pv = []
            for kt in range(NKT):
                jd = kt // 4
                for p in range(jd // 2, 2):
                    js = [j for j in (2 * p, 2 * p + 1) if j >= jd]
                    w = 512 * len(js)
                    # columns left of the causal diagonal are never read:
                    # skip them in the exp
                    coff = 128 * (kt % 4) if js[0] == jd else 0
                    sc = psum.tile([128, 1024], f32, tag="sc", bufs=2,
                                   name=f"sc_h{h}_{kt}_{p}")
                    for i, j in enumerate(js):
                        # for the diagonal q-block, columns left of the causal
                        # front (q < 128*kt) are fully masked: skip them entirely
                        c0 = coff if j == jd else 0
                        nc.tensor.matmul(
                            sc[:, 512 * i + c0:512 * (i + 1)],
                            kc[:, 128 * kt:128 * (kt + 1)],
                            qc[:, 512 * j + c0:512 * (j + 1)],
                            start=True, stop=True)
                    pt = ptp.tile([128, 1024], bf16, tag="pt", name=f"pt_h{h}_{kt}_{p}")
                    nc.scalar.activation(pt[:, coff:w], sc[:, coff:w], Exp, scale=0.125)
                    if js[0] == jd:
                        # diagonal tile: zero out entries above the diagonal;
                        # with the coff shift: keep iff f_local >= p
                        nc.gpsimd.affine_select(
                            out=pt[:, coff:512], in_=pt[:, coff:512],
                            compare_op=mybir.AluOpType.is_ge, fill=0.0,
                            base=0,
                            pattern=[[1, 512 - coff]], channel_multiplier=-1)
                    def emit_pv(kt=kt, js=js, coff=coff, pt=pt, jd=jd, h=h,
                                ctx_t=ctx_t, cs=cs, lr=lr):
                        for i, j in enumerate(js):
                            c0 = coff if j == jd else 0
                            nc.tensor.matmul(
                                ctx_t[j][:, c0:512],
                                va_sb[kt][:, (HD + 1) * h:(HD + 1) * (h + 1)],
                                pt[:, 512 * i + c0:512 * (i + 1)],
                                start=(kt == 0), stop=(kt == 4 * j + 3))
                            if kt == 4 * j + 3:
                                # accumulation done: evacuate PSUM promptly so
                                # the bank frees for the next head's PV group;
                                # spread the row-sum row l across 32 partitions
                                # for the reciprocal
                                nc.vector.tensor_copy(cs[j][:], ctx_t[j][:])
                                nc.sync.dma_start(lr[32 * j:32 * (j + 1), :],
                                                  cs[j][64:65, :])
                    pending_pv.append(emit_pv)
                    if len(pending_pv) > 1:
                        pending_pv.pop(0)()
            for fn in pending_pv:
                fn()
            # epilogue: normalize by the row sums l.
            li = epp.tile([128, 16], f32, tag="li", name=f"li{h}")
            nc.vector.reciprocal(li[:], lr[:])
            ld2 = ldp.tile([1, S], f32, tag="ld2", name=f"ld2_{h}")
            nc.sync.dma_start(ld2.rearrange("a (p c) -> (a p) c", c=16), li[:])
            for j in range(4):
                lbc = bcp.tile([64, 512], f32, tag="lbc", name=f"lbc{h}_{j}")
                nc.sync.dma_start(
                    lbc[:],
                    ld2[0:1, 512 * j:512 * (j + 1)].to_broadcast((64, 512)))
                # DVE supports partition-shifted output APs (verified on HW),
                # so odd heads write rows 64:128 directly
                ro = 64 * (h % 2)
                nc.vector.tensor_mul(
                    ctxn_sb[chk][ro:ro + 64, 512 * j:512 * (j + 1)],
                    cs[j][0:64, :], lbc[:])

        # ---- Phase 3: output projection ----
        for st in range(NST):
            for eb in range(2):
                ps = psum.tile([128, 512], f32, tag="sc", bufs=2, name=f"po{st}_{eb}")
                for c in range(ET):
                    nc.tensor.matmul(
                        ps[:],
                        ctxn_sb[c][:, 128 * st:128 * (st + 1)],
                        wo_sb[c][:, 512 * eb:512 * (eb + 1)],
                        start=(c == 0), stop=(c == ET - 1))
                ot = otp.tile([128, 512], f32, tag="ot", name=f"ot{st}_{eb}")
                nc.vector.tensor_copy(ot[:], ps[:])
                nc.sync.dma_start(
                    out[128 * st:128 * (st + 1), 512 * eb:512 * (eb + 1)], ot[:])

    nc.compile()
    return nc


def _get_nc():
    if "nc" not in _NC_CACHE:
        _NC_CACHE["nc"] = _build_nc()
    return _NC_CACHE["nc"]


def kernel(x, Wq, Wk, Wv, Wo, bo):
    from concourse import bass_utils

    x = np.asarray(x, dtype=np.float32)
    Wq = np.asarray(Wq, dtype=np.float32)
    Wk = np.asarray(Wk, dtype=np.float32)
    Wv = np.asarray(Wv, dtype=np.float32)
    Wo = np.asarray(Wo, dtype=np.float32)
    bo = np.asarray(bo, dtype=np.float32)

    bf = ml_dtypes.bfloat16
    in_maps = []
    for core in range(8):
        b, g = core // 2, core % 2
        sl = slice(E * g, E * (g + 1))
        in_maps.append({
            "xT": np.ascontiguousarray(x[b].T).astype(bf),
            "wqT": np.ascontiguousarray(Wq[sl, :].T).astype(bf),
            "wkT": np.ascontiguousarray(Wk[sl, :].T).astype(bf),
            "wvT": np.ascontiguousarray(Wv[sl, :].T).astype(bf),
            "woT": np.ascontiguousarray(Wo[:, sl].T).astype(bf),
        })

    nc = _get_nc()
    res = bass_utils.run_bass_kernel_spmd(nc, in_maps, core_ids=list(range(8)))
    parts = [r["out"] for r in res.results]
    full = np.empty((4, S, D), np.float32)
    for b in range(4):
        full[b] = parts[2 * b] + parts[2 * b + 1] + bo
    return full


# revision 19
# speedup vs baseline: 1.0623x; 1.0623x over previous
"""Multi-head causal attention (B=4, S=2048, D=1024, H=16) on 8 NeuronCores.

Sharding: batch x head-group. Core c handles batch c//2 and heads
[8*(c%2), 8*(c%2)+8). Wq/Wk/Wv are split column-wise by head (512 output
features per core), Wo row-wise; the 2-way partial-sum reduction over
head-groups (+ bias) is done on the host after gathering per-core outputs.

Per-core dataflow (all matmul operands bf16, fp32 PSUM accumulation):
  QT[e,s], KT[e,s] = Wslice^T-stationary x^T-moving matmuls (transposed layouts)
  V[s,e] natural layout, augmented with a ones column per head (row sums)
  scoresT[k,q] = KT-slice stationary, QT moving  (K=64 contraction per head)
  pT = exp(scoresT/8)  (no max subtraction: logits are bounded ~|2.5|)
  causal masking via gpsimd affine_select on diagonal tiles (fill 0 post-exp)
  ctx^T[hd,q] (+ row-sum l in row 64) = Vaug stationary, pT moving
  normalize by 1/l (DMA partition-broadcast of reciprocal), out-proj with Wo^T.
"""

import contextlib

import numpy as np
import ml_dtypes

S = 2048
D = 1024
E = 512       # per-core output features of Q/K/V projections (8 heads x 64)
HL = 8        # local heads
HD = 64
DC = D // 128   # 8 contraction chunks for projections
ET = E // 128   # 4 chunks for QT/KT/ctxn
NSB = S // 512  # 4 s-blocks
NST = S // 128  # 16 s-tiles
NKT = S // 128  # 16 k-tiles

_NC_CACHE = {}


def _build_nc():
    import concourse.tile as tile
    from concourse import bacc, mybir

    f32 = mybir.dt.float32
    bf16 = mybir.dt.bfloat16
    Exp = mybir.ActivationFunctionType.Exp

    nc = bacc.Bacc("TRN2", target_bir_lowering=False, debug=False, enable_asserts=False)
    xT = nc.dram_tensor("xT", [D, S], bf16, kind="ExternalInput").ap()
    wqT = nc.dram_tensor("wqT", [D, E], bf16, kind="ExternalInput").ap()
    wkT = nc.dram_tensor("wkT", [D, E], bf16, kind="ExternalInput").ap()
    wvT = nc.dram_tensor("wvT", [D, E], bf16, kind="ExternalInput").ap()
    woT = nc.dram_tensor("woT", [E, D], bf16, kind="ExternalInput").ap()
    out = nc.dram_tensor("out", [S, D], f32, kind="ExternalOutput").ap()

    with tile.TileContext(nc) as tc, contextlib.ExitStack() as ctx:
        const = ctx.enter_context(tc.tile_pool(name="const", bufs=1))

        xT_sb = [const.tile([128, S], bf16, tag=f"xT{c}", name=f"xT{c}") for c in range(DC)]
        wq_sb = [const.tile([128, E], bf16, tag=f"wq{c}", name=f"wq{c}") for c in range(DC)]
        wk_sb = [const.tile([128, E], bf16, tag=f"wk{c}", name=f"wk{c}") for c in range(DC)]
        wv_sb = [const.tile([128, E], bf16, tag=f"wv{c}", name=f"wv{c}") for c in range(DC)]
        wo_sb = [const.tile([128, D], bf16, tag=f"wo{c}", name=f"wo{c}") for c in range(ET)]
        qT_sb = [const.tile([128, S], bf16, tag=f"qT{c}", name=f"qT{c}") for c in range(ET)]
        # K^T in two zero-padded parity variants: kTe has the odd head's rows
        # zeroed (and vice versa) so scores matmuls contract over K=128
        # partitions (K=64 matmuls stream at half rate on TRN2).
        kTe_sb = [const.tile([128, S], bf16, tag=f"kTe{c}", name=f"kTe{c}") for c in range(ET)]
        kTo_sb = [const.tile([128, S], bf16, tag=f"kTo{c}", name=f"kTo{c}") for c in range(ET)]
        # V augmented: per s-tile, 8 heads x (64 V cols + 1 ones col)
        va_sb = [const.tile([128, HL * (HD + 1)], bf16, tag=f"va{t}", name=f"va{t}")
                 for t in range(NST)]
        ctxn_sb = [const.tile([128, S], bf16, tag=f"cn{c}", name=f"cn{c}") for c in range(ET)]

        # load order tracks first use: xT s-block 0 and wq feed the very first
        # projection group; wo is needed last
        for c in range(DC):
            nc.sync.dma_start(xT_sb[c][:, 0:512], xT[128 * c:128 * (c + 1), 0:512])
        for c in range(DC):
            nc.sync.dma_start(wq_sb[c][:], wqT[128 * c:128 * (c + 1), :])
        for c in range(DC):
            nc.sync.dma_start(wk_sb[c][:], wkT[128 * c:128 * (c + 1), :])
        for c in range(DC):
            nc.sync.dma_start(wv_sb[c][:], wvT[128 * c:128 * (c + 1), :])
        for sb in range(1, NSB):
            for c in range(DC):
                nc.sync.dma_start(xT_sb[c][:, 512 * sb:512 * (sb + 1)],
                                  xT[128 * c:128 * (c + 1), 512 * sb:512 * (sb + 1)])
        for c in range(ET):
            nc.sync.dma_start(wo_sb[c][:], woT[128 * c:128 * (c + 1), :])
        for t in range(NST):
            # ones columns at 64 + 65*k within each 130-wide head pair group
            va4 = va_sb[t].rearrange("p (g k c) -> p g k c", k=2, c=HD + 1)
            nc.vector.memset(va4[:, :, :, HD:HD + 1], 1.0)
        for c in range(ET):
            # zero the dead halves once; phase 1 fills only the live halves
            nc.vector.memset(kTe_sb[c][64:128, :], 0.0)
            nc.vector.memset(kTo_sb[c][0:64, :], 0.0)

        # One PSUM pool for all phases: tag "sc" slots ([128,1024] = 2 banks,
        # bufs=2) are shared by projection, scores, and out-projection tiles;
        # tags ctx0..3 hold the per-q-block PV accumulators (1 bank each).
        # 2*2 + 4 = 8 banks, no pool-boundary serialization between phases.
        psum = ctx.enter_context(tc.tile_pool(name="psum", bufs=1, space="PSUM"))
        ptp = ctx.enter_context(tc.tile_pool(name="ptp", bufs=8))
        csp = ctx.enter_context(tc.tile_pool(name="csp", bufs=8))
        epp = ctx.enter_context(tc.tile_pool(name="epp", bufs=2))
        bcp = ctx.enter_context(tc.tile_pool(name="bcp", bufs=3))
        ldp = ctx.enter_context(tc.tile_pool(name="ldp", bufs=2, space="DRAM"))
        stp = ctx.enter_context(tc.tile_pool(name="stp", bufs=3))
        otp = ctx.enter_context(tc.tile_pool(name="otp", bufs=4))

        # ---- phase-1 work-unit emitters ----
        def emit_qk_group(is_k, et, sb, ptag="sc", pbufs=2):
            w_sb = wk_sb if is_k else wq_sb
            ps = psum.tile([128, 512], f32, tag=ptag, bufs=pbufs,
                           name=f"p{'k' if is_k else 'q'}{et}_{sb}_{ptag}")
            for dc in range(DC):
                nc.tensor.matmul(
                    ps[:],
                    w_sb[dc][:, 128 * et:128 * (et + 1)],
                    xT_sb[dc][:, 512 * sb:512 * (sb + 1)],
                    start=(dc == 0), stop=(dc == DC - 1))
            ssl = slice(512 * sb, 512 * (sb + 1))
            if is_k:
                nc.vector.tensor_copy(kTe_sb[et][0:64, ssl], ps[0:64, :])
                nc.vector.tensor_copy(kTo_sb[et][64:128, ssl], ps[64:128, :])
            else:
                nc.vector.tensor_copy(qT_sb[et][:, ssl], ps[:])

        def emit_v_group(st, ptag="sc", pbufs=2):
            ps = psum.tile([128, 512], f32, tag=ptag, bufs=pbufs,
                           name=f"pv{st}_{ptag}")
            for dc in range(DC):
                nc.tensor.matmul(
                    ps[:],
                    xT_sb[dc][:, 128 * st:128 * (st + 1)],
                    wv_sb[dc][:],
                    start=(dc == 0), stop=(dc == DC - 1))
            va = va_sb[st].rearrange("p (g pair) -> p g pair", pair=2 * (HD + 1))
            psg = ps.rearrange("p (g c) -> p g c", c=2 * HD)
            nc.vector.tensor_copy(va[:, :, 0:HD], psg[:, :, 0:HD])
            nc.vector.tensor_copy(va[:, :, HD + 1:2 * HD + 1], psg[:, :, HD:2 * HD])

        # ---- Phase 1 upfront: only what head 0 needs to get going ----
        # (Q/K chunk 0, V s-tiles 0..7); the rest is injected into the
        # attention instruction stream where the PE would otherwise stall.
        emit_qk_group(False, 0, 0)
        emit_qk_group(True, 0, 0)
        for st in range(4):
            emit_v_group(st)
        for sb in range(1, NSB):
            emit_qk_group(False, 0, sb)
            emit_qk_group(True, 0, sb)
            if sb == 1:
                for st in range(4, 8):
                    emit_v_group(st)

        # injection queues: head h consumes inj[h] one group per chunk, starting
        # after its j=0 ctx accumulator has been evacuated (the injected groups
        # borrow the ctx0 PSUM bank)
        inj = {h: [] for h in range(HL)}
        inj[0] = [(emit_v_group, (st,)) for st in range(8, 16)] + \
                 [(emit_qk_group, (False, 1, sb)) for sb in range(NSB)]
        inj[1] = [(emit_qk_group, (True, 1, sb)) for sb in range(NSB)]
        inj[2] = [(emit_qk_group, (False, 2, sb)) for sb in range(NSB)] + \
                 [(emit_qk_group, (True, 2, sb)) for sb in range(2)]
        inj[3] = [(emit_qk_group, (True, 2, sb)) for sb in range(2, 4)]
        inj[4] = [(emit_qk_group, (False, 3, sb)) for sb in range(NSB)] + \
                 [(emit_qk_group, (True, 3, sb)) for sb in range(2)]
        inj[5] = [(emit_qk_group, (True, 3, sb)) for sb in range(2, 4)]

        # ---- Phase 2: attention per head ----
        for h in range(HL):
            chk = h // 2
            kc = (kTo_sb if h % 2 else kTe_sb)[chk]
            qc = qT_sb[chk]
            ctx_t = [psum.tile([65, 512], f32, tag=f"ctx{j}", name=f"ctx_h{h}_{j}")
                     for j in range(4)]
            cs = [csp.tile([65, 512], f32, tag="cs", name=f"cs_h{h}_{j}")
                  for j in range(4)]
            lr = epp.tile([128, 16], f32, tag="lr", name=f"lr{h}")
            # software pipeline: PV matmuls for a chunk are emitted after the
            # NEXT chunk's scores, so the in-order PE queue has score work to
            # run while ACT computes the exp the PV depends on
            pending_pv = []
            chunk_i = 0
            for kt in range(NKT):
                jd = kt // 4
                for p in range(jd // 2, 2):
                    if chunk_i >= 9 and inj[h]:
                        f, args = inj[h].pop(0)
                        f(*args, ptag="ctx0", pbufs=1)
                    chunk_i += 1
                    js = [j for j in (2 * p, 2 * p + 1) if j >= jd]
                    w = 512 * len(js)
                    # columns left of the causal diagonal are never read:
                    # skip them in the exp
                    coff = 128 * (kt % 4) if js[0] == jd else 0
                    sc = psum.tile([128, 1024], f32, tag="sc", bufs=2,
                                   name=f"sc_h{h}_{kt}_{p}")
                    for i, j in enumerate(js):
                        # for the diagonal q-block, columns left of the causal
                        # front (q < 128*kt) are fully masked: skip them entirely
                        c0 = coff if j == jd else 0
                        nc.tensor.matmul(
                            sc[:, 512 * i + c0:512 * (i + 1)],
                            kc[:, 128 * kt:128 * (kt + 1)],
                            qc[:, 512 * j + c0:512 * (j + 1)],
                            start=True, stop=True)
                    pt = ptp.tile([128, 1024], bf16, tag="pt", name=f"pt_h{h}_{kt}_{p}")
                    nc.scalar.activation(pt[:, coff:w], sc[:, coff:w], Exp, scale=0.125)
                    if js[0] == jd:
                        # diagonal tile: zero out entries above the diagonal;
                        # with the coff shift: keep iff f_local >= p
                        nc.gpsimd.affine_select(
                            out=pt[:, coff:512], in_=pt[:, coff:512],
                            compare_op=mybir.AluOpType.is_ge, fill=0.0,
                            base=0,
                            pattern=[[1, 512 - coff]], channel_multiplier=-1)
                    def emit_pv(kt=kt, js=js, coff=coff, pt=pt, jd=jd, h=h,
                                ctx_t=ctx_t, cs=cs, lr=lr):
                        for i, j in enumerate(js):
                            c0 = coff if j == jd else 0
                            nc.tensor.matmul(
                                ctx_t[j][:, c0:512],
                                va_sb[kt][:, (HD + 1) * h:(HD + 1) * (h + 1)],
                                pt[:, 512 * i + c0:512 * (i + 1)],
                                start=(kt == 0), stop=(kt == 4 * j + 3))
                            if kt == 4 * j + 3:
                                # accumulation done: evacuate PSUM promptly so
                                # the bank frees for the next head's PV group;
                                # spread the row-sum row l across 32 partitions
                                # for the reciprocal
                                nc.vector.tensor_copy(cs[j][:], ctx_t[j][:])
                                nc.sync.dma_start(lr[32 * j:32 * (j + 1), :],
                                                  cs[j][64:65, :])
                    pending_pv.append(emit_pv)
                    if len(pending_pv) > 1:
                        pending_pv.pop(0)()
            for fn in pending_pv:
                fn()
            # epilogue: normalize by the row sums l.
            li = epp.tile([128, 16], f32, tag="li", name=f"li{h}")
            nc.vector.reciprocal(li[:], lr[:])
            ld2 = ldp.tile([1, S], f32, tag="ld2", name=f"ld2_{h}")
            nc.sync.dma_start(ld2.rearrange("a (p c) -> (a p) c", c=16), li[:])
            for j in range(4):
                lbc = bcp.tile([64, 512], f32, tag="lbc", name=f"lbc{h}_{j}")
                nc.sync.dma_start(
                    lbc[:],
                    ld2[0:1, 512 * j:512 * (j + 1)].to_broadcast((64, 512)))
                # DVE supports partition-shifted output APs (verified on HW),
                # so odd heads write rows 64:128 directly
                ro = 64 * (h % 2)
                nc.vector.tensor_mul(
                    ctxn_sb[chk][ro:ro + 64, 512 * j:512 * (j + 1)],
                    cs[j][0:64, :], lbc[:])

        # ---- Phase 3: output projection ----
        for st in range(NST):
            for eb in range(2):
                ps = psum.tile([128, 512], f32, tag="sc", bufs=2, name=f"po{st}_{eb}")
                for c in range(ET):
                    nc.tensor.matmul(
                        ps[:],
                        ctxn_sb[c][:, 128 * st:128 * (st + 1)],
                        wo_sb[c][:, 512 * eb:512 * (eb + 1)],
                        start=(c == 0), stop=(c == ET - 1))
                ot = otp.tile([128, 512], f32, tag="ot", name=f"ot{st}_{eb}")
                nc.vector.tensor_copy(ot[:], ps[:])
                nc.sync.dma_start(
                    out[128 * st:128 * (st + 1), 512 * eb:512 * (eb + 1)], ot[:])

    nc.compile()
    return nc


def _get_nc():
    if "nc" not in _NC_CACHE:
        _NC_CACHE["nc"] = _build_nc()
    return _NC_CACHE["nc"]


def kernel(x, Wq, Wk, Wv, Wo, bo):
    from concourse import bass_utils

    x = np.asarray(x, dtype=np.float32)
    Wq = np.asarray(Wq, dtype=np.float32)
    Wk = np.asarray(Wk, dtype=np.float32)
    Wv = np.asarray(Wv, dtype=np.float32)
    Wo = np.asarray(Wo, dtype=np.float32)
    bo = np.asarray(bo, dtype=np.float32)

    bf = ml_dtypes.bfloat16
    in_maps = []
    for core in range(8):
        b, g = core // 2, core % 2
        sl = slice(E * g, E * (g + 1))
        in_maps.append({
            "xT": np.ascontiguousarray(x[b].T).astype(bf),
            "wqT": np.ascontiguousarray(Wq[sl, :].T).astype(bf),
            "wkT": np.ascontiguousarray(Wk[sl, :].T).astype(bf),
            "wvT": np.ascontiguousarray(Wv[sl, :].T).astype(bf),
            "woT": np.ascontiguousarray(Wo[:, sl].T).astype(bf),
        })

    nc = _get_nc()
    res = bass_utils.run_bass_kernel_spmd(nc, in_maps, core_ids=list(range(8)))
    parts = [r["out"] for r in res.results]
    full = np.empty((4, S, D), np.float32)
    for b in range(4):
        full[b] = parts[2 * b] + parts[2 * b + 1] + bo
    return full


# revision 22
# speedup vs baseline: 1.1779x; 1.1088x over previous
"""Multi-head causal attention (B=4, S=2048, D=1024, H=16) on 8 NeuronCores.

Sharding: batch x head-group. Core c handles batch c//2 and heads
[8*(c%2), 8*(c%2)+8). Wq/Wk/Wv are split column-wise by head (512 output
features per core), Wo row-wise; the 2-way partial-sum reduction over
head-groups (+ bias) is done on the host after gathering per-core outputs.

Per-core dataflow (all matmul operands bf16, fp32 PSUM accumulation):
  QT[e,s], KT[e,s] = Wslice^T-stationary x^T-moving matmuls (transposed layouts)
  V[s,e] natural layout, augmented with a ones column per head (row sums)
  scoresT[k,q] = KT-slice stationary, QT moving  (K=64 contraction per head)
  pT = exp(scoresT/8)  (no max subtraction: logits are bounded ~|2.5|)
  causal masking via gpsimd affine_select on diagonal tiles (fill 0 post-exp)
  ctx^T[hd,q] (+ row-sum l in row 64) = Vaug stationary, pT moving
  normalize by 1/l (DMA partition-broadcast of reciprocal), out-proj with Wo^T.
"""

import contextlib

import numpy as np
import ml_dtypes

S = 2048
D = 1024
E = 512       # per-core output features of Q/K/V projections (8 heads x 64)
HL = 8        # local heads
HD = 64
DC = D // 128   # 8 contraction chunks for projections
ET = E // 128   # 4 chunks for QT/KT/ctxn
NSB = S // 512  # 4 s-blocks
NST = S // 128  # 16 s-tiles
NKT = S // 128  # 16 k-tiles

_NC_CACHE = {}


def _build_nc():
    import concourse.tile as tile
    from concourse import bacc, mybir

    f32 = mybir.dt.float32
    bf16 = mybir.dt.bfloat16
    Exp = mybir.ActivationFunctionType.Exp

    nc = bacc.Bacc("TRN2", target_bir_lowering=False, debug=False, enable_asserts=False)
    xT = nc.dram_tensor("xT", [D, S], bf16, kind="ExternalInput").ap()
    wqT = nc.dram_tensor("wqT", [D, E], bf16, kind="ExternalInput").ap()
    wkT = nc.dram_tensor("wkT", [D, E], bf16, kind="ExternalInput").ap()
    wvT = nc.dram_tensor("wvT", [D, E], bf16, kind="ExternalInput").ap()
    woT = nc.dram_tensor("woT", [E, D], bf16, kind="ExternalInput").ap()
    out = nc.dram_tensor("out", [S, D], f32, kind="ExternalOutput").ap()

    with tile.TileContext(nc) as tc, contextlib.ExitStack() as ctx:
        const = ctx.enter_context(tc.tile_pool(name="const", bufs=1))

        xT_sb = [const.tile([128, S], bf16, tag=f"xT{c}", name=f"xT{c}") for c in range(DC)]
        wq_sb = [const.tile([128, E], bf16, tag=f"wq{c}", name=f"wq{c}") for c in range(DC)]
        wk_sb = [const.tile([128, E], bf16, tag=f"wk{c}", name=f"wk{c}") for c in range(DC)]
        wv_sb = [const.tile([128, E], bf16, tag=f"wv{c}", name=f"wv{c}") for c in range(DC)]
        wo_sb = [const.tile([128, D], bf16, tag=f"wo{c}", name=f"wo{c}") for c in range(ET)]
        qT_sb = [const.tile([128, S], bf16, tag=f"qT{c}", name=f"qT{c}") for c in range(ET)]
        # K^T in two zero-padded parity variants: kTe has the odd head's rows
        # zeroed (and vice versa) so scores matmuls contract over K=128
        # partitions (K=64 matmuls stream at half rate on TRN2).
        kTe_sb = [const.tile([128, S], bf16, tag=f"kTe{c}", name=f"kTe{c}") for c in range(ET)]
        kTo_sb = [const.tile([128, S], bf16, tag=f"kTo{c}", name=f"kTo{c}") for c in range(ET)]
        # V augmented: per s-tile, 8 heads x (64 V cols + 1 ones col)
        va_sb = [const.tile([128, HL * (HD + 1)], bf16, tag=f"va{t}", name=f"va{t}")
                 for t in range(NST)]
        ctxn_sb = [const.tile([128, S], bf16, tag=f"cn{c}", name=f"cn{c}") for c in range(ET)]

        # load order tracks first use: xT s-block 0 and wq feed the very first
        # projection group (interleaved so matmul dc=0 can start after two
        # small DMAs); wo is needed last
        for c in range(DC):
            nc.sync.dma_start(xT_sb[c][:, 0:512], xT[128 * c:128 * (c + 1), 0:512])
            nc.sync.dma_start(wq_sb[c][:], wqT[128 * c:128 * (c + 1), :])
        for c in range(DC):
            nc.sync.dma_start(wk_sb[c][:], wkT[128 * c:128 * (c + 1), :])
        for c in range(DC):
            nc.sync.dma_start(wv_sb[c][:], wvT[128 * c:128 * (c + 1), :])
        for sb in range(1, NSB):
            for c in range(DC):
                nc.sync.dma_start(xT_sb[c][:, 512 * sb:512 * (sb + 1)],
                                  xT[128 * c:128 * (c + 1), 512 * sb:512 * (sb + 1)])
        for c in range(ET):
            nc.sync.dma_start(wo_sb[c][:], woT[128 * c:128 * (c + 1), :])
        for t in range(NST):
            # ones columns at 64 + 65*k within each 130-wide head pair group
            va4 = va_sb[t].rearrange("p (g k c) -> p g k c", k=2, c=HD + 1)
            nc.vector.memset(va4[:, :, :, HD:HD + 1], 1.0)
        for c in range(ET):
            # zero the dead halves once; phase 1 fills only the live halves
            nc.vector.memset(kTe_sb[c][64:128, :], 0.0)
            nc.vector.memset(kTo_sb[c][0:64, :], 0.0)

        # One PSUM pool for all phases: tag "sc" slots ([128,1024] = 2 banks,
        # bufs=2) are shared by projection, scores, and out-projection tiles;
        # tags ctx0..3 hold the per-q-block PV accumulators (1 bank each).
        # 2*2 + 4 = 8 banks, no pool-boundary serialization between phases.
        psum = ctx.enter_context(tc.tile_pool(name="psum", bufs=1, space="PSUM"))
        ptp = ctx.enter_context(tc.tile_pool(name="ptp", bufs=8))
        csp = ctx.enter_context(tc.tile_pool(name="csp", bufs=8))
        epp = ctx.enter_context(tc.tile_pool(name="epp", bufs=2))
        bcp = ctx.enter_context(tc.tile_pool(name="bcp", bufs=3))
        ldp = ctx.enter_context(tc.tile_pool(name="ldp", bufs=2, space="DRAM"))
        stp = ctx.enter_context(tc.tile_pool(name="stp", bufs=3))
        otp = ctx.enter_context(tc.tile_pool(name="otp", bufs=4))

        # ---- phase-1 work-unit emitters ----
        def emit_qk_group(is_k, et, sb, ptag="sc", pbufs=2):
            w_sb = wk_sb if is_k else wq_sb
            ps = psum.tile([128, 512], f32, tag=ptag, bufs=pbufs,
                           name=f"p{'k' if is_k else 'q'}{et}_{sb}_{ptag}")
            for dc in range(DC):
                nc.tensor.matmul(
                    ps[:],
                    w_sb[dc][:, 128 * et:128 * (et + 1)],
                    xT_sb[dc][:, 512 * sb:512 * (sb + 1)],
                    start=(dc == 0), stop=(dc == DC - 1))
            ssl = slice(512 * sb, 512 * (sb + 1))
            if is_k:
                nc.vector.tensor_copy(kTe_sb[et][0:64, ssl], ps[0:64, :])
                nc.vector.tensor_copy(kTo_sb[et][64:128, ssl], ps[64:128, :])
            else:
                nc.vector.tensor_copy(qT_sb[et][:, ssl], ps[:])

        def emit_v_group(st, ptag="sc", pbufs=2):
            ps = psum.tile([128, 512], f32, tag=ptag, bufs=pbufs,
                           name=f"pv{st}_{ptag}")
            for dc in range(DC):
                nc.tensor.matmul(
                    ps[:],
                    xT_sb[dc][:, 128 * st:128 * (st + 1)],
                    wv_sb[dc][:],
                    start=(dc == 0), stop=(dc == DC - 1))
            va = va_sb[st].rearrange("p (g pair) -> p g pair", pair=2 * (HD + 1))
            psg = ps.rearrange("p (g c) -> p g c", c=2 * HD)
            nc.vector.tensor_copy(va[:, :, 0:HD], psg[:, :, 0:HD])
            nc.vector.tensor_copy(va[:, :, HD + 1:2 * HD + 1], psg[:, :, HD:2 * HD])

        # ---- Phase 1 upfront: only what head 0 needs to get going ----
        # (Q/K chunk 0, V s-tiles 0..7); the rest is injected into the
        # attention instruction stream where the PE would otherwise stall.
        emit_qk_group(False, 0, 0)
        emit_qk_group(True, 0, 0)
        for st in range(4):
            emit_v_group(st)
        for sb in range(1, NSB):
            emit_qk_group(False, 0, sb)
            emit_qk_group(True, 0, sb)
            if sb == 1:
                for st in range(4, 8):
                    emit_v_group(st)

        # injection queues: head h consumes inj[h] one group per chunk, starting
        # after its j=0 ctx accumulator has been evacuated (the injected groups
        # borrow the ctx0 PSUM bank)
        inj = {h: [] for h in range(HL)}
        inj[0] = [(emit_v_group, (st,)) for st in range(8, 16)] + \
                 [(emit_qk_group, (False, 1, sb)) for sb in range(NSB)]
        inj[1] = [(emit_qk_group, (True, 1, sb)) for sb in range(NSB)]
        inj[2] = [(emit_qk_group, (False, 2, sb)) for sb in range(NSB)] + \
                 [(emit_qk_group, (True, 2, sb)) for sb in range(2)]
        inj[3] = [(emit_qk_group, (True, 2, sb)) for sb in range(2, 4)]
        inj[4] = [(emit_qk_group, (False, 3, sb)) for sb in range(NSB)] + \
                 [(emit_qk_group, (True, 3, sb)) for sb in range(2)]
        inj[5] = [(emit_qk_group, (True, 3, sb)) for sb in range(2, 4)]

        # ---- Phase 2: attention per head ----
        for h in range(HL):
            chk = h // 2
            kc = (kTo_sb if h % 2 else kTe_sb)[chk]
            qc = qT_sb[chk]
            ctx_t = [psum.tile([65, 512], f32, tag=f"ctx{j}", name=f"ctx_h{h}_{j}")
                     for j in range(4)]
            cs = [csp.tile([65, 512], f32, tag="cs", name=f"cs_h{h}_{j}")
                  for j in range(4)]
            lr = epp.tile([128, 16], f32, tag="lr", name=f"lr{h}")
            # software pipeline: PV matmuls for a chunk are emitted after the
            # NEXT chunk's scores, so the in-order PE queue has score work to
            # run while ACT computes the exp the PV depends on
            pending_pv = []
            chunk_i = 0
            for kt in range(NKT):
                jd = kt // 4
                for p in range(jd // 2, 2):
                    if chunk_i >= 9 and inj[h]:
                        f, args = inj[h].pop(0)
                        f(*args, ptag="ctx0", pbufs=1)
                    chunk_i += 1
                    js = [j for j in (2 * p, 2 * p + 1) if j >= jd]
                    w = 512 * len(js)
                    # columns left of the causal diagonal are never read:
                    # skip them in the exp
                    coff = 128 * (kt % 4) if js[0] == jd else 0
                    sc = psum.tile([128, 1024], f32, tag="sc", bufs=2,
                                   name=f"sc_h{h}_{kt}_{p}")
                    for i, j in enumerate(js):
                        # for the diagonal q-block, columns left of the causal
                        # front (q < 128*kt) are fully masked: skip them entirely
                        c0 = coff if j == jd else 0
                        nc.tensor.matmul(
                            sc[:, 512 * i + c0:512 * (i + 1)],
                            kc[:, 128 * kt:128 * (kt + 1)],
                            qc[:, 512 * j + c0:512 * (j + 1)],
                            start=True, stop=True)
                    pt = ptp.tile([128, 1024], bf16, tag="pt", name=f"pt_h{h}_{kt}_{p}")
                    nc.scalar.activation(pt[:, coff:w], sc[:, coff:w], Exp, scale=0.125)
                    if js[0] == jd:
                        # diagonal tile: zero out entries above the diagonal;
                        # with the coff shift: keep iff f_local >= p
                        nc.gpsimd.affine_select(
                            out=pt[:, coff:512], in_=pt[:, coff:512],
                            compare_op=mybir.AluOpType.is_ge, fill=0.0,
                            base=0,
                            pattern=[[1, 512 - coff]], channel_multiplier=-1)
                    def emit_pv(kt=kt, js=js, coff=coff, pt=pt, jd=jd, h=h,
                                ctx_t=ctx_t, cs=cs, lr=lr):
                        for i, j in enumerate(js):
                            c0 = coff if j == jd else 0
                            nc.tensor.matmul(
                                ctx_t[j][:, c0:512],
                                va_sb[kt][:, (HD + 1) * h:(HD + 1) * (h + 1)],
                                pt[:, 512 * i + c0:512 * (i + 1)],
                                start=(kt == 0), stop=(kt == 4 * j + 3))
                            if kt == 4 * j + 3:
                                # accumulation done: evacuate PSUM promptly so
                                # the bank frees for the next head's PV group;
                                # spread the row-sum row l across 32 partitions
                                # for the reciprocal
                                nc.vector.tensor_copy(cs[j][:], ctx_t[j][:])
                                nc.sync.dma_start(lr[32 * j:32 * (j + 1), :],
                                                  cs[j][64:65, :])
                    pending_pv.append(emit_pv)
                    if len(pending_pv) > 2:
                        pending_pv.pop(0)()
            for fn in pending_pv:
                fn()
            # epilogue: normalize by the row sums l.
            li = epp.tile([128, 16], f32, tag="li", name=f"li{h}")
            nc.vector.reciprocal(li[:], lr[:])
            ld2 = ldp.tile([1, S], f32, tag="ld2", name=f"ld2_{h}")
            nc.sync.dma_start(ld2.rearrange("a (p c) -> (a p) c", c=16), li[:])
            for j in range(4):
                lbc = bcp.tile([64, 512], f32, tag="lbc", name=f"lbc{h}_{j}")
                nc.sync.dma_start(
                    lbc[:],
                    ld2[0:1, 512 * j:512 * (j + 1)].to_broadcast((64, 512)))
                # DVE supports partition-shifted output APs (verified on HW),
                # so odd heads write rows 64:128 directly
                ro = 64 * (h % 2)
                nc.vector.tensor_mul(
                    ctxn_sb[chk][ro:ro + 64, 512 * j:512 * (j + 1)],
                    cs[j][0:64, :], lbc[:])

        # ---- Phase 3: output projection ----
        for st in range(NST):
            for eb in range(2):
                ps = psum.tile([128, 512], f32, tag="sc", bufs=2, name=f"po{st}_{eb}")
                for c in range(ET):
                    nc.tensor.matmul(
                        ps[:],
                        ctxn_sb[c][:, 128 * st:128 * (st + 1)],
                        wo_sb[c][:, 512 * eb:512 * (eb + 1)],
                        start=(c == 0), stop=(c == ET - 1))
                ot = otp.tile([128, 512], f32, tag="ot", name=f"ot{st}_{eb}")
                nc.vector.tensor_copy(ot[:], ps[:])
                nc.sync.dma_start(
                    out[128 * st:128 * (st + 1), 512 * eb:512 * (eb + 1)], ot[:])

    nc.compile()
    return nc


def _get_nc():
    if "nc" not in _NC_CACHE:
        _NC_CACHE["nc"] = _build_nc()
    return _NC_CACHE["nc"]


def kernel(x, Wq, Wk, Wv, Wo, bo):
    from concourse import bass_utils

    x = np.asarray(x, dtype=np.float32)
    Wq = np.asarray(Wq, dtype=np.float32)
    Wk = np.asarray(Wk, dtype=np.float32)
    Wv = np.asarray(Wv, dtype=np.float32)
    Wo = np.asarray(Wo, dtype=np.float32)
    bo = np.asarray(bo, dtype=np.float32)

    bf = ml_dtypes.bfloat16
    in_maps = []
    for core in range(8):
        b, g = core // 2, core % 2
        sl = slice(E * g, E * (g + 1))
        in_maps.append({
            "xT": np.ascontiguousarray(x[b].T).astype(bf),
            "wqT": np.ascontiguousarray(Wq[sl, :].T).astype(bf),
            "wkT": np.ascontiguousarray(Wk[sl, :].T).astype(bf),
            "wvT": np.ascontiguousarray(Wv[sl, :].T).astype(bf),
            "woT": np.ascontiguousarray(Wo[:, sl].T).astype(bf),
        })

    nc = _get_nc()
    res = None
    for attempt in range(3):
        try:
            res = bass_utils.run_bass_kernel_spmd(nc, in_maps, core_ids=list(range(8)))
            break
        except Exception:
            if attempt == 2:
                raise
    assert res is not None
    parts = [r["out"] for r in res.results]
    full = np.empty((4, S, D), np.float32)
    for b in range(4):
        full[b] = parts[2 * b] + parts[2 * b + 1] + bo
    return full


# revision 28
# speedup vs baseline: 1.1924x; 1.0123x over previous
"""Multi-head causal attention (B=4, S=2048, D=1024, H=16) on 8 NeuronCores.

Sharding: batch x head-group. Core c handles batch c//2 and heads
[8*(c%2), 8*(c%2)+8). Wq/Wk/Wv are split column-wise by head (512 output
features per core), Wo row-wise; the 2-way partial-sum reduction over
head-groups (+ bias) is done on the host after gathering per-core outputs.

Per-core dataflow (all matmul operands bf16, fp32 PSUM accumulation):
  QT[e,s], KT[e,s] = Wslice^T-stationary x^T-moving matmuls (transposed layouts)
  V[s,e] natural layout, augmented with a ones column per head (row sums)
  scoresT[k,q] = KT-slice stationary, QT moving  (K=64 contraction per head)
  pT = exp(scoresT/8)  (no max subtraction: logits are bounded ~|2.5|)
  causal masking via gpsimd affine_select on diagonal tiles (fill 0 post-exp)
  ctx^T[hd,q] (+ row-sum l in row 64) = Vaug stationary, pT moving
  normalize by 1/l (DMA partition-broadcast of reciprocal), out-proj with Wo^T.
"""

import contextlib

import numpy as np
import ml_dtypes

S = 2048
D = 1024
E = 512       # per-core output features of Q/K/V projections (8 heads x 64)
HL = 8        # local heads
HD = 64
DC = D // 128   # 8 contraction chunks for projections
ET = E // 128   # 4 chunks for QT/KT/ctxn
NSB = S // 512  # 4 s-blocks
NST = S // 128  # 16 s-tiles
NKT = S // 128  # 16 k-tiles

_NC_CACHE = {}


def _build_nc():
    import concourse.tile as tile
    from concourse import bacc, mybir

    f32 = mybir.dt.float32
    bf16 = mybir.dt.bfloat16
    Exp = mybir.ActivationFunctionType.Exp

    nc = bacc.Bacc("TRN2", target_bir_lowering=False, debug=False, enable_asserts=False)
    xT = nc.dram_tensor("xT", [D, S], bf16, kind="ExternalInput").ap()
    wqT = nc.dram_tensor("wqT", [D, E], bf16, kind="ExternalInput").ap()
    wkT = nc.dram_tensor("wkT", [D, E], bf16, kind="ExternalInput").ap()
    wvT = nc.dram_tensor("wvT", [D, E], bf16, kind="ExternalInput").ap()
    woT = nc.dram_tensor("woT", [E, D], bf16, kind="ExternalInput").ap()
    out = nc.dram_tensor("out", [S, D], f32, kind="ExternalOutput").ap()

    with tile.TileContext(nc) as tc, contextlib.ExitStack() as ctx:
        const = ctx.enter_context(tc.tile_pool(name="const", bufs=1))
        # x^T and the QKV weights are only needed until the last injected
        # projection group (end of head 3); this pool is released then so the
        # out-projection partial tiles can reuse the space
        xwp = tc.alloc_tile_pool(name="xwp", bufs=1, side="right")

        xT_sb = [xwp.tile([128, S], bf16, tag=f"xT{c}", name=f"xT{c}") for c in range(DC)]
        wq_sb = [xwp.tile([128, E], bf16, tag=f"wq{c}", name=f"wq{c}") for c in range(DC)]
        wk_sb = [xwp.tile([128, E], bf16, tag=f"wk{c}", name=f"wk{c}") for c in range(DC)]
        wv_sb = [xwp.tile([128, E], bf16, tag=f"wv{c}", name=f"wv{c}") for c in range(DC)]
        wo_sb = [const.tile([128, D], bf16, tag=f"wo{c}", name=f"wo{c}") for c in range(ET)]
        qT_sb = [const.tile([128, S], bf16, tag=f"qT{c}", name=f"qT{c}") for c in range(ET)]
        # K^T in two zero-padded parity variants: kTe has the odd head's rows
        # zeroed (and vice versa) so scores matmuls contract over K=128
        # partitions (K=64 matmuls stream at half rate on TRN2).
        kTe_sb = [const.tile([128, S], bf16, tag=f"kTe{c}", name=f"kTe{c}") for c in range(ET)]
        kTo_sb = [const.tile([128, S], bf16, tag=f"kTo{c}", name=f"kTo{c}") for c in range(ET)]
        # V augmented: per s-tile, 8 heads x (64 V cols + 1 ones col)
        va_sb = [const.tile([128, HL * (HD + 1)], bf16, tag=f"va{t}", name=f"va{t}")
                 for t in range(NST)]
        ctxn_sb = [const.tile([128, S], bf16, tag=f"cn{c}", name=f"cn{c}") for c in range(ET)]

        # load order tracks first use: xT s-block 0 and wq feed the very first
        # projection group (interleaved so matmul dc=0 can start after two
        # small DMAs); wo is needed last
        for c in range(DC):
            nc.sync.dma_start(xT_sb[c][:, 0:512], xT[128 * c:128 * (c + 1), 0:512])
            nc.sync.dma_start(wq_sb[c][:], wqT[128 * c:128 * (c + 1), :])
        for c in range(DC):
            nc.sync.dma_start(wk_sb[c][:], wkT[128 * c:128 * (c + 1), :])
        for c in range(DC):
            nc.sync.dma_start(wv_sb[c][:], wvT[128 * c:128 * (c + 1), :])
        for sb in range(1, NSB):
            for c in range(DC):
                nc.sync.dma_start(xT_sb[c][:, 512 * sb:512 * (sb + 1)],
                                  xT[128 * c:128 * (c + 1), 512 * sb:512 * (sb + 1)])
        for c in range(ET):
            nc.sync.dma_start(wo_sb[c][:], woT[128 * c:128 * (c + 1), :])
        for t in range(NST):
            # ones columns at 64 + 65*k within each 130-wide head pair group
            va4 = va_sb[t].rearrange("p (g k c) -> p g k c", k=2, c=HD + 1)
            nc.vector.memset(va4[:, :, :, HD:HD + 1], 1.0)
        for c in range(ET):
            # zero the dead halves once; phase 1 fills only the live halves
            nc.vector.memset(kTe_sb[c][64:128, :], 0.0)
            nc.vector.memset(kTo_sb[c][0:64, :], 0.0)

        # One PSUM pool for all phases: tag "sc" slots ([128,1024] = 2 banks,
        # bufs=2) are shared by projection, scores, and out-projection tiles;
        # tags ctx0..3 hold the per-q-block PV accumulators (1 bank each).
        # 2*2 + 4 = 8 banks, no pool-boundary serialization between phases.
        psum = ctx.enter_context(tc.tile_pool(name="psum", bufs=1, space="PSUM"))
        ptp = ctx.enter_context(tc.tile_pool(name="ptp", bufs=8))
        csp = ctx.enter_context(tc.tile_pool(name="csp", bufs=8))
        epp = ctx.enter_context(tc.tile_pool(name="epp", bufs=2))
        bcp = ctx.enter_context(tc.tile_pool(name="bcp", bufs=3))
        ldp = ctx.enter_context(tc.tile_pool(name="ldp", bufs=2, space="DRAM"))
        stp = ctx.enter_context(tc.tile_pool(name="stp", bufs=3))
        otp = ctx.enter_context(tc.tile_pool(name="otp", bufs=4))

        # ---- phase-1 work-unit emitters ----
        def emit_qk_group(is_k, et, sb, ptag="sc", pbufs=2):
            w_sb = wk_sb if is_k else wq_sb
            ps = psum.tile([128, 512], f32, tag=ptag, bufs=pbufs,
                           name=f"p{'k' if is_k else 'q'}{et}_{sb}_{ptag}")
            for dc in range(DC):
                nc.tensor.matmul(
                    ps[:],
                    w_sb[dc][:, 128 * et:128 * (et + 1)],
                    xT_sb[dc][:, 512 * sb:512 * (sb + 1)],
                    start=(dc == 0), stop=(dc == DC - 1))
            ssl = slice(512 * sb, 512 * (sb + 1))
            if is_k:
                nc.vector.tensor_copy(kTe_sb[et][0:64, ssl], ps[0:64, :])
                nc.vector.tensor_copy(kTo_sb[et][64:128, ssl], ps[64:128, :])
            else:
                nc.vector.tensor_copy(qT_sb[et][:, ssl], ps[:])

        def emit_v_group(st, ptag="sc", pbufs=2):
            ps = psum.tile([128, 512], f32, tag=ptag, bufs=pbufs,
                           name=f"pv{st}_{ptag}")
            for dc in range(DC):
                nc.tensor.matmul(
                    ps[:],
                    xT_sb[dc][:, 128 * st:128 * (st + 1)],
                    wv_sb[dc][:],
                    start=(dc == 0), stop=(dc == DC - 1))
            va = va_sb[st].rearrange("p (g pair) -> p g pair", pair=2 * (HD + 1))
            psg = ps.rearrange("p (g c) -> p g c", c=2 * HD)
            nc.vector.tensor_copy(va[:, :, 0:HD], psg[:, :, 0:HD])
            nc.vector.tensor_copy(va[:, :, HD + 1:2 * HD + 1], psg[:, :, HD:2 * HD])

        # ---- Phase 1 upfront: only what head 0 needs to get going ----
        # (Q/K chunk 0, V s-tiles 0..7); the rest is injected into the
        # attention instruction stream where the PE would otherwise stall.
        emit_qk_group(False, 0, 0)
        emit_qk_group(True, 0, 0)
        for st in range(4):
            emit_v_group(st)
        for sb in range(1, NSB):
            emit_qk_group(False, 0, sb)
            emit_qk_group(True, 0, sb)
            if sb == 1:
                for st in range(4, 8):
                    emit_v_group(st)

        # out-projection partial over ctxn chunks 0-1 (heads 0-3); injected into
        # heads 4-6 so only the c2+c3 half remains after the last head
        p01_sb = {}

        def emit_p01(st, eb, ptag="ctx0", pbufs=1):
            ps = psum.tile([128, 512], f32, tag=ptag, bufs=pbufs, name=f"pp{st}_{eb}")
            for c in (0, 1):
                nc.tensor.matmul(
                    ps[:],
                    ctxn_sb[c][:, 128 * st:128 * (st + 1)],
                    wo_sb[c][:, 512 * eb:512 * (eb + 1)],
                    start=(c == 0), stop=(c == 1))
            t = ppp.tile([128, 512], bf16, tag=f"p01_{st}_{eb}", name=f"p01_{st}_{eb}")
            p01_sb[(st, eb)] = t
            nc.vector.tensor_copy(t[:], ps[:])

        # injection queues: head h consumes inj[h] one group per chunk, starting
        # after its j=0 ctx accumulator has been evacuated (the injected groups
        # borrow the ctx0 PSUM bank)
        inj = {h: [] for h in range(HL)}
        inj[0] = [(emit_v_group, (st,)) for st in range(8, 16)] + \
                 [(emit_qk_group, (False, 1, sb)) for sb in range(NSB)]
        inj[1] = [(emit_qk_group, (True, 1, sb)) for sb in range(NSB)]
        inj[2] = [(emit_qk_group, (False, 2, sb)) for sb in range(NSB)] + \
                 [(emit_qk_group, (True, 2, sb)) for sb in range(NSB)] + \
                 [(emit_qk_group, (False, 3, sb)) for sb in range(3)]
        inj[3] = [(emit_qk_group, (False, 3, 3))] + \
                 [(emit_qk_group, (True, 3, sb)) for sb in range(NSB)]
        p01_items = [(emit_p01, (st, eb)) for st in range(NST) for eb in range(2)]
        inj[4] = p01_items[0:15]
        inj[5] = p01_items[15:30]
        inj[6] = p01_items[30:32]

        # ---- Phase 2: attention per head ----
        ppp = None
        for h in range(HL):
            chk = h // 2
            kc = (kTo_sb if h % 2 else kTe_sb)[chk]
            qc = qT_sb[chk]
            ctx_t = [psum.tile([65, 512], f32, tag=f"ctx{j}", name=f"ctx_h{h}_{j}")
                     for j in range(4)]
            cs = [csp.tile([65, 512], f32, tag="cs", name=f"cs_h{h}_{j}")
                  for j in range(4)]
            ld2 = ldp.tile([1, S], f32, tag="ld2", name=f"ld2_{h}")
            ld2_r = ld2.rearrange("a (p c) -> (a p) c", c=16)
            # software pipeline: PV matmuls for a chunk are emitted after the
            # NEXT chunk's scores, so the in-order PE queue has score work to
            # run while ACT computes the exp the PV depends on
            pending_pv = []
            chunk_i = 0
            for kt in range(NKT):
                jd = kt // 4
                for p in range(jd // 2, 2):
                    if chunk_i >= 9 and inj[h]:
                        f, args = inj[h].pop(0)
                        f(*args, ptag="ctx0", pbufs=1)
                    chunk_i += 1
                    js = [j for j in (2 * p, 2 * p + 1) if j >= jd]
                    w = 512 * len(js)
                    # columns left of the causal diagonal are never read:
                    # skip them in the exp
                    coff = 128 * (kt % 4) if js[0] == jd else 0
                    sc = psum.tile([128, 1024], f32, tag="sc", bufs=2,
                                   name=f"sc_h{h}_{kt}_{p}")
                    for i, j in enumerate(js):
                        # for the diagonal q-block, columns left of the causal
                        # front (q < 128*kt) are fully masked: skip them entirely
                        c0 = coff if j == jd else 0
                        nc.tensor.matmul(
                            sc[:, 512 * i + c0:512 * (i + 1)],
                            kc[:, 128 * kt:128 * (kt + 1)],
                            qc[:, 512 * j + c0:512 * (j + 1)],
                            start=True, stop=True)
                    pt = ptp.tile([128, 1024], bf16, tag="pt", name=f"pt_h{h}_{kt}_{p}")
                    nc.scalar.activation(pt[:, coff:w], sc[:, coff:w], Exp, scale=0.125)
                    if js[0] == jd:
                        # diagonal tile: zero out entries above the diagonal;
                        # with the coff shift: keep iff f_local >= p
                        nc.gpsimd.affine_select(
                            out=pt[:, coff:512], in_=pt[:, coff:512],
                            compare_op=mybir.AluOpType.is_ge, fill=0.0,
                            base=0,
                            pattern=[[1, 512 - coff]], channel_multiplier=-1)
                    def emit_pv(kt=kt, js=js, coff=coff, pt=pt, jd=jd, h=h,
                                chk=chk, ctx_t=ctx_t, cs=cs, ld2=ld2, ld2_r=ld2_r):
                        for i, j in enumerate(js):
                            c0 = coff if j == jd else 0
                            nc.tensor.matmul(
                                ctx_t[j][:, c0:512],
                                va_sb[kt][:, (HD + 1) * h:(HD + 1) * (h + 1)],
                                pt[:, 512 * i + c0:512 * (i + 1)],
                                start=(kt == 0), stop=(kt == 4 * j + 3))
                            if kt == 4 * j + 3:
                                # accumulation done: evacuate PSUM promptly so
                                # the bank frees for the next head's PV group,
                                # then run this q-block's whole normalize chain:
                                # spread l over 32 partitions, reciprocal, DRAM
                                # bounce, partition-broadcast, multiply
                                nc.vector.tensor_copy(cs[j][:], ctx_t[j][:])
                                lrj = epp.tile([32, 16], f32, tag="lrj", bufs=8,
                                               name=f"lr{h}_{j}")
                                nc.sync.dma_start(lrj[:], cs[j][64:65, :])
                                lij = epp.tile([32, 16], f32, tag="lij", bufs=8,
                                               name=f"li{h}_{j}")
                                nc.vector.reciprocal(lij[:], lrj[:])
                                nc.sync.dma_start(ld2_r[32 * j:32 * (j + 1), :],
                                                  lij[:])
                                lbc = bcp.tile([64, 512], f32, tag="lbc",
                                               name=f"lbc{h}_{j}")
                                nc.sync.dma_start(
                                    lbc[:],
                                    ld2[0:1, 512 * j:512 * (j + 1)]
                                    .to_broadcast((64, 512)))
                                ro = 64 * (h % 2)
                                nc.vector.tensor_mul(
                                    ctxn_sb[chk][ro:ro + 64, 512 * j:512 * (j + 1)],
                                    cs[j][0:64, :], lbc[:])
                    pending_pv.append(emit_pv)
                    if len(pending_pv) > 2:
                        pending_pv.pop(0)()
            for fn in pending_pv:
                fn()
            if h == 3:
                # x^T / Wq / Wk / Wv are fully consumed; free their SBUF so the
                # out-projection partial tiles can live there
                xwp.release()
                ppp = tc.alloc_tile_pool(name="ppp", bufs=1, side="right")

        # ---- Phase 3: output projection (c2+c3 half; c0+c1 was injected) ----
        for st in range(NST):
            for eb in range(2):
                ps = psum.tile([128, 512], f32, tag="sc", bufs=2, name=f"po{st}_{eb}")
                for c in (2, 3):
                    nc.tensor.matmul(
                        ps[:],
                        ctxn_sb[c][:, 128 * st:128 * (st + 1)],
                        wo_sb[c][:, 512 * eb:512 * (eb + 1)],
                        start=(c == 2), stop=(c == 3))
                ot = otp.tile([128, 512], f32, tag="ot", name=f"ot{st}_{eb}")
                nc.vector.tensor_add(ot[:], ps[:], p01_sb[(st, eb)][:])
                nc.sync.dma_start(
                    out[128 * st:128 * (st + 1), 512 * eb:512 * (eb + 1)], ot[:])
        ppp.release()

    nc.compile()
    return nc


def _get_nc():
    if "nc" not in _NC_CACHE:
        _NC_CACHE["nc"] = _build_nc()
    return _NC_CACHE["nc"]


def kernel(x, Wq, Wk, Wv, Wo, bo):
    from concourse import bass_utils

    x = np.asarray(x, dtype=np.float32)
    Wq = np.asarray(Wq, dtype=np.float32)
    Wk = np.asarray(Wk, dtype=np.float32)
    Wv = np.asarray(Wv, dtype=np.float32)
    Wo = np.asarray(Wo, dtype=np.float32)
    bo = np.asarray(bo, dtype=np.float32)

    bf = ml_dtypes.bfloat16
    in_maps = []
    for core in range(8):
        b, g = core // 2, core % 2
        sl = slice(E * g, E * (g + 1))
        in_maps.append({
            "xT": np.ascontiguousarray(x[b].T).astype(bf),
            "wqT": np.ascontiguousarray(Wq[sl, :].T).astype(bf),
            "wkT": np.ascontiguousarray(Wk[sl, :].T).astype(bf),
            "wvT": np.ascontiguousarray(Wv[sl, :].T).astype(bf),
            "woT": np.ascontiguousarray(Wo[:, sl].T).astype(bf),
        })

    nc = _get_nc()
    res = None
    for attempt in range(3):
        try:
            res = bass_utils.run_bass_kernel_spmd(nc, in_maps, core_ids=list(range(8)))
            break
        except Exception:
            if attempt == 2:
                raise
    assert res is not None
    parts = [r["out"] for r in res.results]
    full = np.empty((4, S, D), np.float32)
    for b in range(4):
        full[b] = parts[2 * b] + parts[2 * b + 1] + bo
    return full
